# revision 1
# baseline (speedup 1.0000x reference)
"""Trainium2 Bass kernel for nn_DepthwiseCrossViTMAE (criss-cross multihead self-attention).

Reference computation per token t (B*L = 4096 tokens, hidden 2048 = C*K with C=32, K=64):
  qkv[c, :] = x[t, c*64:(c+1)*64] @ Wqkv[c] + bqkv[c]          (per-channel linear)
  q, k, v = split(qkv)                                          each (C, K)
  for each d in [0, 64):  S_d = outer(q[:, d], k[:, d]) / 8     (C x C)
                          A_d = softmax_rows(S_d)
                          ctx[d, m] = sum_c A_d[c, m] * v[c, d]
  out[t, c*64:(c+1)*64] = ctx.T[c] @ Wout + bout

Sharding: data-parallel over the 4096 tokens, 512 tokens per core on 8 cores.

Per-core layout: SBUF partitions p = (t2, d) with t2 in {0,1}, d in [0,64);
token t = t2*256 + tau.  q/k/v live as [p, (tau, c)].  The scores tensor is
built per chunk of Tc tau's as [p, (tau, c, e)] by a broadcast tensor_tensor
outer product on DVE, exp on ACT, row-sums (over e) and weighted column-sums
(over c) as segmented tensor_reduce on DVE.  PE does the QKV projections
(contracting k=64 per channel), the x transposes, and the output projection.
"""

import sys

sys.path.insert(0, "/opt/trn_rl_repo")

import numpy as np
from contextlib import ExitStack

import concourse.bass as bass
import concourse.bacc as bacc
import concourse.mybir as mybir
import concourse.tile as tile
from concourse.masks import make_identity

F32 = mybir.dt.float32
AF = mybir.ActivationFunctionType
ALU = mybir.AluOpType
AX = mybir.AxisListType

C = 32          # channels
K = 64          # per-channel width (also Wout dim)
F = C * K       # hidden = 2048
N_CORES = 8


def build_nc(T=512, Tc=2, flush=8, gp_num=13, gp_den=16):
    """Build the single-core Bass module for T tokens.

    Tc: tau-chunk size for the attention loop (free size per DVE op = Tc*1024).
    flush: chunks per output-DMA flush group.
    gp_num/gp_den: chunks with ch % gp_den < gp_num run their elementwise
    score/P passes on GPSIMD instead of DVE (engine load balancing).
    """
    T2 = 2
    TH = T // T2            # tokens per half (tau range)
    NFC = F // 128          # 16 feature chunks of x
    NTT = (T + 127) // 128  # token tiles of x
    NCH = TH // Tc          # attention chunks
    assert TH % Tc == 0 and NCH % flush == 0

    nc = bacc.Bacc()
    x_d = nc.dram_tensor("x", [T, F], F32, kind="ExternalInput")
    wq_d = nc.dram_tensor("wq", [F, K], F32, kind="ExternalInput")
    wk_d = nc.dram_tensor("wk", [F, K], F32, kind="ExternalInput")
    wv_d = nc.dram_tensor("wv", [F, K], F32, kind="ExternalInput")
    bq_d = nc.dram_tensor("bq", [128, C], F32, kind="ExternalInput")
    bk_d = nc.dram_tensor("bk", [128, C], F32, kind="ExternalInput")
    bv_d = nc.dram_tensor("bv", [128, C], F32, kind="ExternalInput")
    wo_d = nc.dram_tensor("wout", [128, K], F32, kind="ExternalInput")
    bo_d = nc.dram_tensor("bout", [128, K], F32, kind="ExternalInput")
    out_d = nc.dram_tensor("out", [T, F], F32, kind="ExternalOutput")

    with ExitStack() as octx:
        tc = octx.enter_context(tile.TileContext(nc))
        const_pool = octx.enter_context(tc.tile_pool(name="const", bufs=1))
        qkv_pool = octx.enter_context(tc.tile_pool(name="qkv", bufs=1))

        ident = const_pool.tile([128, 128], F32)
        make_identity(nc, ident[:])
        ones_row = const_pool.tile([1, 128], F32)
        nc.gpsimd.memset(ones_row[:], 1.0)

        bq_sb = const_pool.tile([128, C], F32)
        bk_sb = const_pool.tile([128, C], F32)
        bv_sb = const_pool.tile([128, C], F32)
        wo_sb = const_pool.tile([128, K], F32)
        bo_sb = const_pool.tile([128, K], F32)
        nc.sync.dma_start(bq_sb[:], bq_d[:])
        nc.sync.dma_start(bk_sb[:], bk_d[:])
        nc.sync.dma_start(bv_sb[:], bv_d[:])
        nc.sync.dma_start(wo_sb[:], wo_d[:])
        nc.sync.dma_start(bo_sb[:], bo_d[:])

        # q is pre-scaled by 1/sqrt(K) on the host (weights and bias).
        qs_sb = qkv_pool.tile([128, TH, C], F32)
        k_sb = qkv_pool.tile([128, TH, C], F32)
        v_sb = qkv_pool.tile([128, TH, C], F32)

        # ---------------- phase 1: load x, transpose, QKV projections ----
        with (
            tc.tile_pool(name="xload", bufs=2) as xpool,
            tc.tile_pool(name="xt", bufs=1) as xtpool,
            tc.tile_pool(name="wgt", bufs=1) as wpool,
            tc.tile_pool(name="ps_qkv", bufs=6, space="PSUM") as ps1,
            tc.tile_pool(name="ps_tr", bufs=2, space="PSUM") as pst,
        ):
            wq_sb = wpool.tile([128, NFC, K], F32)
            wk_sb = wpool.tile([128, NFC, K], F32)
            wv_sb = wpool.tile([128, NFC, K], F32)
            nc.sync.dma_start(wq_sb[:], wq_d[:].rearrange("(fc p) d -> p fc d", p=128))
            nc.sync.dma_start(wk_sb[:], wk_d[:].rearrange("(fc p) d -> p fc d", p=128))
            nc.sync.dma_start(wv_sb[:], wv_d[:].rearrange("(fc p) d -> p fc d", p=128))

            # xT[feat, token] per 128-feature chunk, via PE transpose.
            # Token-tile order 0,2,1,3 puts both t2-halves of the earliest
            # taus first so the attention loop can start sooner.
            xt_sb = xtpool.tile([128, NFC, T], F32)
            tt_order = [0, 2, 1, 3] if NTT == 4 else list(range(NTT))
            for tt in tt_order:
                trows = min(128, T - tt * 128)
                x_sb = xpool.tile([128, F], F32)
                for fc in range(NFC):
                    nc.sync.dma_start(
                        x_sb[:trows, fc * 128 : (fc + 1) * 128],
                        x_d[tt * 128 : tt * 128 + trows, fc * 128 : (fc + 1) * 128],
                    )
                for fc in range(NFC):
                    ps_t = pst.tile([128, 128], F32)
                    nc.tensor.transpose(
                        ps_t[:, :trows],
                        x_sb[:trows, fc * 128 : (fc + 1) * 128],
                        ident[:trows, :trows],
                    )
                    nc.vector.tensor_copy(
                        xt_sb[:, fc, tt * 128 : tt * 128 + trows], ps_t[:, :trows]
                    )

            # per-channel QKV projections, split into tau-halves so the
            # attention loop can start after the first half is done
            NQ = 2
            HH = TH // NQ
            for half in range(NQ):
                hsl = slice(half * HH, half * HH + HH)
                for c in range(C):
                    fc, h = divmod(c, 2)
                    hp = slice(64 * h, 64 * h + 64)
                    for qkv_i, (w_sb, b_sb, dst) in enumerate(
                        (
                            (wq_sb, bq_sb, qs_sb),
                            (wk_sb, bk_sb, k_sb),
                            (wv_sb, bv_sb, v_sb),
                        )
                    ):
                        ps = ps1.tile([128, HH], F32)
                        for t2 in range(T2):
                            nc.tensor.matmul(
                                ps[64 * t2 : 64 * t2 + 64, :],
                                w_sb[hp, fc, :],
                                xt_sb[
                                    hp, fc, t2 * TH + half * HH : t2 * TH + half * HH + HH
                                ],
                                start=True,
                                stop=True,
                            )
                        # all qkv biases on the startup-idle DVE so ACT can
                        # begin the first chunks' exp sooner
                        nc.vector.tensor_tensor(
                            dst[:, hsl, c],
                            ps[:],
                            b_sb[:, c : c + 1].broadcast_to([128, HH]),
                            ALU.add,
                        )

        # ---------------- phase 2: criss-cross attention + out-proj ------
        with (
            tc.tile_pool(name="s", bufs=6) as s_pool,
            tc.tile_pool(name="e", bufs=4) as e_pool,
            tc.tile_pool(name="zsm", bufs=8) as z_pool,
            tc.tile_pool(name="ctx", bufs=6) as ctx_pool,
            tc.tile_pool(name="stage", bufs=2) as stage_pool,
            tc.tile_pool(name="ps_o", bufs=6, space="PSUM") as ps2,
        ):
            stage = [None, None]
            ctx_t = None
            for ch in range(NCH):
                g = ch % flush
                if g == 0:
                    stage = [
                        stage_pool.tile(
                            [Tc * C, flush, K], F32, tag="st0", name="st0"
                        ),
                        stage_pool.tile(
                            [Tc * C, flush, K], F32, tag="st1", name="st1"
                        ),
                    ]
                tsl = slice(ch * Tc, (ch + 1) * Tc)
                # 13/16 of chunks on GPSIMD, spread evenly (not in bursts) so
                # both engines stay fed; keep the tail on DVE (shorter chain)
                on_gp = (ch % gp_den) not in (5, 10, 15) and ch < NCH - 6
                ew = nc.gpsimd if on_gp else nc.vector
                qs4 = qs_sb[:, tsl, :].unsqueeze(3).broadcast_to([128, Tc, C, C])
                k4 = k_sb[:, tsl, :].unsqueeze(2).broadcast_to([128, Tc, C, C])
                s_t = s_pool.tile([128, Tc, C, C], F32)
                ew.tensor_tensor(s_t[:], qs4, k4, ALU.mult)

                e_t = e_pool.tile([128, Tc, C, C], F32)
                nc.scalar.activation(e_t[:], s_t[:], AF.Exp)

                z_t = z_pool.tile([128, Tc, C], F32, tag="z")
                nc.vector.tensor_reduce(z_t[:], e_t[:], AX.X, ALU.add)
                zi_t = z_pool.tile([128, Tc, C], F32, tag="zi")
                nc.vector.reciprocal_approx_fast(zi_t[:], z_t[:])
                w_t = z_pool.tile([128, Tc, C], F32, tag="w")
                ew.tensor_tensor(w_t[:], v_sb[:, tsl, :], zi_t[:], ALU.mult)

                w4 = w_t[:].unsqueeze(3).broadcast_to([128, Tc, C, C])
                ctx_t = ctx_pool.tile([128, Tc, C], F32, tag="ctx", name="ctx")
                ctx_a = ctx_t[:]
                if on_gp:
                    # GPSIMD pays no stride penalty: write P transposed
                    # ([p, tau, m, c]) so the DVE c-reduce is contiguous.
                    ew.tensor_tensor(
                        s_t[:].transpose([0, 1, 3, 2]), e_t[:], w4, ALU.mult
                    )
                    nc.vector.tensor_reduce(ctx_a, s_t[:], AX.X, ALU.add)
                else:
                    # DVE pays 1.6x for any strided op: keep P contiguous
                    # ([p, tau, c, m]) and reduce over c with a contiguous
                    # pairwise tree (sum halves over the middle axis).
                    nc.vector.tensor_tensor(s_t[:], e_t[:], w4, ALU.mult)
                    src = s_t[:]
                    width = C
                    while width > 1:
                        half = width // 2
                        lo = src[:, :, 0:half, :]
                        hi = src[:, :, half:width, :]
                        if half == 1:
                            nc.vector.tensor_tensor(
                                ctx_a, lo.squeeze(2), hi.squeeze(2), ALU.add
                            )
                        else:
                            nc.vector.tensor_tensor(lo, lo, hi, ALU.add)
                        width = half

                # out-proj: out[tau, m, o] = sum_d ctx[(t2,d),(tau,m)] * Wout[d, o]
                # bout folded in via a K=1 accumulating matmul of ones x bout
                for t2 in range(T2):
                    dp = slice(64 * t2, 64 * t2 + 64)
                    po = ps2.tile([Tc * C, K], F32)
                    nc.tensor.matmul(
                        po[:],
                        ctx_t[dp, :, :].rearrange("p t c -> p (t c)"),
                        wo_sb[dp, :],
                        start=True,
                        stop=False,
                    )
                    nc.tensor.matmul(
                        po[:],
                        ones_row[0:1, 0 : Tc * C],
                        bo_sb[0:1, :],
                        start=False,
                        stop=True,
                    )
                    nc.scalar.copy(stage[t2][:, g, :], po[:])

                if g == flush - 1:
                    chb = ch // flush
                    ov = out_d[:].rearrange(
                        "(t2 chb chs tau) (m o) -> t2 chb tau m chs o",
                        t2=T2,
                        chb=NCH // flush,
                        chs=flush,
                        tau=Tc,
                        m=C,
                    )
                    for t2 in range(T2):
                        nc.sync.dma_start(ov[t2, chb], stage[t2][:])

    nc.compile()
    return nc


def _host_prep(x, Wqkv, bqkv, Wout, bout):
    x = np.ascontiguousarray(np.asarray(x, dtype=np.float32)).reshape(-1, F)
    Wqkv = np.asarray(Wqkv, dtype=np.float32)
    bqkv = np.asarray(bqkv, dtype=np.float32)
    Wout = np.asarray(Wout, dtype=np.float32)
    bout = np.asarray(bout, dtype=np.float32)
    scale = 1.0 / np.sqrt(K)

    common = {
        "wq": np.ascontiguousarray((Wqkv[:, :, :K] * scale).reshape(F, K)),
        "wk": np.ascontiguousarray(Wqkv[:, :, K : 2 * K].reshape(F, K)),
        "wv": np.ascontiguousarray(Wqkv[:, :, 2 * K :].reshape(F, K)),
        "bq": np.ascontiguousarray(np.tile((bqkv[:, :K] * scale).T, (2, 1))),
        "bk": np.ascontiguousarray(np.tile(bqkv[:, K : 2 * K].T, (2, 1))),
        "bv": np.ascontiguousarray(np.tile(bqkv[:, 2 * K :].T, (2, 1))),
        "wout": np.ascontiguousarray(np.tile(Wout, (2, 1))),
        "bout": np.ascontiguousarray(np.tile(bout[None, :], (128, 1))),
    }
    return x, common


_NC_CACHE = {}


def _get_nc(T):
    if T not in _NC_CACHE:
        _NC_CACHE[T] = build_nc(T=T)
    return _NC_CACHE[T]


def kernel(x, Wqkv, bqkv, Wout, bout, _trace=False):
    from concourse.bass_utils import run_bass_kernel_spmd

    xs, common = _host_prep(x, Wqkv, bqkv, Wout, bout)
    n_tok = xs.shape[0]
    tpc = n_tok // N_CORES
    in_maps = [
        {**common, "x": np.ascontiguousarray(xs[i * tpc : (i + 1) * tpc])}
        for i in range(N_CORES)
    ]
    nc = _get_nc(tpc)
    res = run_bass_kernel_spmd(nc, in_maps, list(range(N_CORES)), trace=_trace)
    out = np.concatenate([res.results[i]["out"] for i in range(N_CORES)], axis=0)
    out = out.reshape(np.asarray(x).shape)
    if _trace:
        kernel.last_results = res
    return out



# revision 3
# speedup vs baseline: 1.1195x; 1.1195x over previous
"""Trainium2 Bass kernel for nn_DepthwiseCrossViTMAE (criss-cross multihead self-attention).

Reference computation per token t (B*L = 4096 tokens, hidden 2048 = C*K with C=32, K=64):
  qkv[c, :] = x[t, c*64:(c+1)*64] @ Wqkv[c] + bqkv[c]          (per-channel linear)
  q, k, v = split(qkv)                                          each (C, K)
  for each d in [0, 64):  S_d = outer(q[:, d], k[:, d]) / 8     (C x C)
                          A_d = softmax_rows(S_d)
                          ctx[d, m] = sum_c A_d[c, m] * v[c, d]
  out[t, c*64:(c+1)*64] = ctx.T[c] @ Wout + bout

Sharding: data-parallel over the 4096 tokens, 512 tokens per core on 8 cores.

Per-core layout: SBUF partitions p = (t2, d) with t2 in {0,1}, d in [0,64);
token t = t2*256 + tau.  q/k/v live as [p, (tau, c)] in bf16.  Scores are
built per chunk of Tc taus as [p, (tau, e, c)] (c innermost so every
elementwise op after the outer product has packed last dims and hits the
DVE 2x mode in bf16):
  s[p,tau,e,c] = q[p,tau,c] * k[p,tau,e]        (DVE or GPSIMD, 1x: k bcast)
  E = exp(s)                                     (ACT)
  z[p,tau,c] = sum_e E    (pairwise tree over the middle axis, bf16 2x)
  w = v * 1/z                                    (small)
  P[p,tau,e,c] = E * w[bcast e]                  (DVE 2x)
  ctx[p,tau,e] = sum_c P  (pairwise tree over the last axis, bf16 2x)
PE does the QKV projections, x transposes and the output projection (all
bf16); ACT applies QKV biases (per-partition bias AP) while moving PSUM ->
SBUF bf16.
"""

import sys

sys.path.insert(0, "/opt/trn_rl_repo")

import numpy as np
from contextlib import ExitStack

import concourse.bass as bass
import concourse.bacc as bacc
import concourse.mybir as mybir
import concourse.tile as tile
from concourse.masks import make_identity

F32 = mybir.dt.float32
BF16 = mybir.dt.bfloat16
AF = mybir.ActivationFunctionType
ALU = mybir.AluOpType
AX = mybir.AxisListType

C = 32          # channels
K = 64          # per-channel width (also Wout dim)
F = C * K       # hidden = 2048
N_CORES = 8


def build_nc(T=512, Tc=8, flush=8, dve_s_chunks=(11, 27)):
    """Build the single-core Bass module for T tokens.

    Tc: tau-chunk size for the attention loop (free size per elementwise op
    = Tc*1024).
    flush: chunks per output-DMA flush group.
    dve_s_chunks: chunk indices whose score outer-product runs on DVE
    instead of GPSIMD (engine load balancing; GPSIMD takes the rest).
    """
    T2 = 2
    TH = T // T2            # tokens per half (tau range)
    NFC = F // 128          # 16 feature chunks of x
    NTT = (T + 127) // 128  # token tiles of x
    NCH = TH // Tc          # attention chunks
    NSUB = Tc // 4          # out-proj sub-blocks (4 taus x 32 m = 128 rows)
    assert TH % Tc == 0 and NCH % flush == 0 and Tc % 4 == 0

    nc = bacc.Bacc()
    x_d = nc.dram_tensor("x", [T, F], BF16, kind="ExternalInput")
    wq_d = nc.dram_tensor("wq", [F, K], BF16, kind="ExternalInput")
    wk_d = nc.dram_tensor("wk", [F, K], BF16, kind="ExternalInput")
    wv_d = nc.dram_tensor("wv", [F, K], BF16, kind="ExternalInput")
    bq_d = nc.dram_tensor("bq", [128, C], F32, kind="ExternalInput")
    bk_d = nc.dram_tensor("bk", [128, C], F32, kind="ExternalInput")
    bv_d = nc.dram_tensor("bv", [128, C], F32, kind="ExternalInput")
    wo_d = nc.dram_tensor("wout", [128, K], BF16, kind="ExternalInput")
    bo_d = nc.dram_tensor("bout", [128, K], BF16, kind="ExternalInput")
    out_d = nc.dram_tensor("out", [T, F], F32, kind="ExternalOutput")

    with ExitStack() as octx, nc.allow_low_precision(reason="bf16 attention"):
        tc = octx.enter_context(tile.TileContext(nc))
        const_pool = octx.enter_context(tc.tile_pool(name="const", bufs=1))
        qkv_pool = octx.enter_context(tc.tile_pool(name="qkv", bufs=1))

        ident = const_pool.tile([128, 128], BF16)
        make_identity(nc, ident[:])
        ones_row = const_pool.tile([1, 128], BF16)
        nc.gpsimd.memset(ones_row[:], 1.0)

        bq_sb = const_pool.tile([128, C], F32)
        bk_sb = const_pool.tile([128, C], F32)
        bv_sb = const_pool.tile([128, C], F32)
        wo_sb = const_pool.tile([128, K], BF16)
        bo_sb = const_pool.tile([128, K], BF16)
        nc.sync.dma_start(bq_sb[:], bq_d[:])
        nc.sync.dma_start(bk_sb[:], bk_d[:])
        nc.sync.dma_start(bv_sb[:], bv_d[:])
        nc.sync.dma_start(wo_sb[:], wo_d[:])
        nc.sync.dma_start(bo_sb[:], bo_d[:])

        # q is pre-scaled by 1/sqrt(K) on the host (weights and bias).
        qs_sb = qkv_pool.tile([128, TH, C], BF16)
        k_sb = qkv_pool.tile([128, TH, C], BF16)
        v_sb = qkv_pool.tile([128, TH, C], BF16)

        # ---------------- phase 1: load x, transpose, QKV projections ----
        with (
            tc.tile_pool(name="xload", bufs=2) as xpool,
            tc.tile_pool(name="xt", bufs=1) as xtpool,
            tc.tile_pool(name="wgt", bufs=1) as wpool,
            tc.tile_pool(name="ps_qkv", bufs=6, space="PSUM") as ps1,
            tc.tile_pool(name="ps_tr", bufs=2, space="PSUM") as pst,
        ):
            wq_sb = wpool.tile([128, NFC, K], BF16)
            wk_sb = wpool.tile([128, NFC, K], BF16)
            wv_sb = wpool.tile([128, NFC, K], BF16)
            nc.sync.dma_start(wq_sb[:], wq_d[:].rearrange("(fc p) d -> p fc d", p=128))
            nc.sync.dma_start(wk_sb[:], wk_d[:].rearrange("(fc p) d -> p fc d", p=128))
            nc.sync.dma_start(wv_sb[:], wv_d[:].rearrange("(fc p) d -> p fc d", p=128))

            # xT[feat, token] per 128-feature chunk, via PE transpose.
            # Token-tile order 0,2,1,3 puts both t2-halves of the earliest
            # taus first so the attention loop can start sooner.
            xt_sb = xtpool.tile([128, NFC, T], BF16)
            tt_order = [0, 2, 1, 3] if NTT == 4 else list(range(NTT))
            for tt in tt_order:
                trows = min(128, T - tt * 128)
                x_sb = xpool.tile([128, F], BF16)
                for fc in range(NFC):
                    nc.sync.dma_start(
                        x_sb[:trows, fc * 128 : (fc + 1) * 128],
                        x_d[tt * 128 : tt * 128 + trows, fc * 128 : (fc + 1) * 128],
                    )
                for fc in range(NFC):
                    ps_t = pst.tile([128, 128], BF16)
                    nc.tensor.transpose(
                        ps_t[:, :trows],
                        x_sb[:trows, fc * 128 : (fc + 1) * 128],
                        ident[:trows, :trows],
                    )
                    nc.vector.tensor_copy(
                        xt_sb[:, fc, tt * 128 : tt * 128 + trows], ps_t[:, :trows]
                    )

            # per-channel QKV projections, split into tau-halves so the
            # attention loop can start after the first half is done
            NQ = 2
            HH = TH // NQ
            for half in range(NQ):
                hsl = slice(half * HH, half * HH + HH)
                for c in range(C):
                    fc, h = divmod(c, 2)
                    hp = slice(64 * h, 64 * h + 64)
                    for qkv_i, (w_sb, b_sb, dst) in enumerate(
                        (
                            (wq_sb, bq_sb, qs_sb),
                            (wk_sb, bk_sb, k_sb),
                            (wv_sb, bv_sb, v_sb),
                        )
                    ):
                        ps = ps1.tile([128, HH], F32)
                        for t2 in range(T2):
                            nc.tensor.matmul(
                                ps[64 * t2 : 64 * t2 + 64, :],
                                w_sb[hp, fc, :],
                                xt_sb[
                                    hp, fc, t2 * TH + half * HH : t2 * TH + half * HH + HH
                                ],
                                start=True,
                                stop=True,
                            )
                        # bias + cast to bf16 on ACT (per-partition bias AP)
                        nc.scalar.activation(
                            dst[:, hsl, c],
                            ps[:],
                            AF.Identity,
                            bias=b_sb[:, c : c + 1],
                        )

        # ---------------- phase 2: criss-cross attention + out-proj ------
        with (
            tc.tile_pool(name="s", bufs=3) as s_pool,
            tc.tile_pool(name="e", bufs=2) as e_pool,
            tc.tile_pool(name="zscr", bufs=2) as zscr_pool,
            tc.tile_pool(name="zsm", bufs=8) as z_pool,
            tc.tile_pool(name="ctx", bufs=4) as ctx_pool,
            tc.tile_pool(name="stage", bufs=2) as stage_pool,
            tc.tile_pool(name="ps_o", bufs=6, space="PSUM") as ps2,
        ):
            stage = [None, None]
            for ch in range(NCH):
                g = ch % flush
                if g == 0:
                    stage = [
                        stage_pool.tile(
                            [128, flush, NSUB, K], F32, tag="st0", name="st0"
                        ),
                        stage_pool.tile(
                            [128, flush, NSUB, K], F32, tag="st1", name="st1"
                        ),
                    ]
                tsl = slice(ch * Tc, (ch + 1) * Tc)
                on_gp = ch not in dve_s_chunks
                ew = nc.gpsimd if on_gp else nc.vector

                # s[p, tau, e, c] = q[p, tau, c] * k[p, tau, e]
                qs4 = qs_sb[:, tsl, :].unsqueeze(2).broadcast_to([128, Tc, C, C])
                k4 = k_sb[:, tsl, :].unsqueeze(3).broadcast_to([128, Tc, C, C])
                s_t = s_pool.tile([128, Tc, C, C], BF16)
                ew.tensor_tensor(s_t[:], qs4, k4, ALU.mult)

                e_t = e_pool.tile([128, Tc, C, C], BF16)
                nc.scalar.activation(e_t[:], s_t[:], AF.Exp)

                # z[p, tau, c] = sum_e E: pairwise tree over the middle axis,
                # first level into scratch (E must survive for P), last level
                # to fp32 for the reciprocal.
                zscr = zscr_pool.tile([128, Tc, C // 2, C], BF16)
                z_t = z_pool.tile([128, Tc, C], F32, tag="z")
                nc.vector.tensor_tensor(
                    zscr[:], e_t[:, :, 0 : C // 2, :], e_t[:, :, C // 2 : C, :],
                    ALU.add,
                )
                width = C // 2
                while width > 1:
                    half = width // 2
                    lo = zscr[:, :, 0:half, :]
                    hi = zscr[:, :, half:width, :]
                    if half == 1:
                        nc.vector.tensor_tensor(
                            z_t[:], lo.squeeze(2), hi.squeeze(2), ALU.add
                        )
                    else:
                        nc.vector.tensor_tensor(lo, lo, hi, ALU.add)
                    width = half

                zi_t = z_pool.tile([128, Tc, C], F32, tag="zi")
                nc.vector.reciprocal_approx_fast(zi_t[:], z_t[:])
                w_t = z_pool.tile([128, Tc, C], BF16, tag="w")
                nc.vector.tensor_tensor(w_t[:], v_sb[:, tsl, :], zi_t[:], ALU.mult)

                # P[p, tau, e, c] = E * w (both packed -> DVE 2x), into s_t
                w4 = w_t[:].unsqueeze(2).broadcast_to([128, Tc, C, C])
                nc.vector.tensor_tensor(s_t[:], e_t[:], w4, ALU.mult)

                # ctx[p, tau, e] = sum_c P: pairwise tree over the last axis
                ctx_t = ctx_pool.tile([128, Tc, C], BF16, tag="ctx", name="ctx")
                width = C
                while width > 1:
                    half = width // 2
                    lo = s_t[:, :, :, 0:half]
                    hi = s_t[:, :, :, half:width]
                    if half == 1:
                        nc.vector.tensor_tensor(
                            ctx_t[:], lo.squeeze(3), hi.squeeze(3), ALU.add
                        )
                    else:
                        nc.vector.tensor_tensor(lo, lo, hi, ALU.add)
                    width = half

                # out-proj: out[(tau4,m), o] = sum_d ctx[(t2,d),(tau,m)] * Wout[d, o]
                # bout folded in via a K=1 accumulating matmul of ones x bout
                for t2 in range(T2):
                    dp = slice(64 * t2, 64 * t2 + 64)
                    for sub in range(NSUB):
                        ssl = slice(sub * 4, sub * 4 + 4)
                        po = ps2.tile([128, K], F32)
                        nc.tensor.matmul(
                            po[:],
                            ctx_t[dp, ssl, :].rearrange("p t c -> p (t c)"),
                            wo_sb[dp, :],
                            start=True,
                            stop=False,
                        )
                        nc.tensor.matmul(
                            po[:],
                            ones_row[0:1, 0:128],
                            bo_sb[0:1, :],
                            start=False,
                            stop=True,
                        )
                        nc.scalar.copy(stage[t2][:, g, sub, :], po[:])

                if g == flush - 1:
                    chb = ch // flush
                    ov = out_d[:].rearrange(
                        "(t2 chb chs sub tau) (m o) -> t2 chb tau m chs sub o",
                        t2=T2,
                        chb=NCH // flush,
                        chs=flush,
                        sub=NSUB,
                        tau=4,
                        m=C,
                    )
                    for t2 in range(T2):
                        nc.sync.dma_start(ov[t2, chb], stage[t2][:])

    nc.compile()
    return nc


def _bf16(a):
    import ml_dtypes

    return np.ascontiguousarray(a.astype(ml_dtypes.bfloat16))


def _host_prep(x, Wqkv, bqkv, Wout, bout):
    x = np.ascontiguousarray(np.asarray(x, dtype=np.float32)).reshape(-1, F)
    Wqkv = np.asarray(Wqkv, dtype=np.float32)
    bqkv = np.asarray(bqkv, dtype=np.float32)
    Wout = np.asarray(Wout, dtype=np.float32)
    bout = np.asarray(bout, dtype=np.float32)
    scale = 1.0 / np.sqrt(K)

    common = {
        "wq": _bf16((Wqkv[:, :, :K] * scale).reshape(F, K)),
        "wk": _bf16(Wqkv[:, :, K : 2 * K].reshape(F, K)),
        "wv": _bf16(Wqkv[:, :, 2 * K :].reshape(F, K)),
        "bq": np.ascontiguousarray(np.tile((bqkv[:, :K] * scale).T, (2, 1))),
        "bk": np.ascontiguousarray(np.tile(bqkv[:, K : 2 * K].T, (2, 1))),
        "bv": np.ascontiguousarray(np.tile(bqkv[:, 2 * K :].T, (2, 1))),
        "wout": _bf16(np.tile(Wout, (2, 1))),
        "bout": _bf16(np.tile(bout[None, :], (128, 1))),
    }
    return _bf16(x), common


_NC_CACHE = {}


def _get_nc(T):
    if T not in _NC_CACHE:
        _NC_CACHE[T] = build_nc(T=T)
    return _NC_CACHE[T]


def kernel(x, Wqkv, bqkv, Wout, bout, _trace=False):
    from concourse.bass_utils import run_bass_kernel_spmd

    xs, common = _host_prep(x, Wqkv, bqkv, Wout, bout)
    n_tok = xs.shape[0]
    tpc = n_tok // N_CORES
    in_maps = [
        {**common, "x": np.ascontiguousarray(xs[i * tpc : (i + 1) * tpc])}
        for i in range(N_CORES)
    ]
    nc = _get_nc(tpc)
    res = run_bass_kernel_spmd(nc, in_maps, list(range(N_CORES)), trace=_trace)
    out = np.concatenate([res.results[i]["out"] for i in range(N_CORES)], axis=0)
    out = out.reshape(np.asarray(x).shape)
    if _trace:
        kernel.last_results = res
    return out


# revision 9
# speedup vs baseline: 1.4842x; 1.3257x over previous
"""Trainium2 Bass kernel for nn_DepthwiseCrossViTMAE (criss-cross multihead self-attention).

Reference computation per token t (B*L = 4096 tokens, hidden 2048 = C*K with C=32, K=64):
  qkv[c, :] = x[t, c*64:(c+1)*64] @ Wqkv[c] + bqkv[c]          (per-channel linear)
  q, k, v = split(qkv)                                          each (C, K)
  for each d in [0, 64):  S_d = outer(q[:, d], k[:, d]) / 8     (C x C)
                          A_d = softmax_rows(S_d)
                          ctx[d, m] = sum_c A_d[c, m] * v[c, d]
  out[t, c*64:(c+1)*64] = ctx.T[c] @ Wout + bout

Sharding: data-parallel over the 4096 tokens, 512 tokens per core on 8 cores.

Per-core layout: SBUF partitions p = (t2, d) with t2 in {0,1}, d in [0,64);
token t = t2*256 + tau.  q/k/v live as [p, (tau, c)] in bf16.  Scores are
built per chunk of Tc taus as [p, (tau, e, c)] (c innermost so every
elementwise op after the outer product has packed last dims and hits the
DVE 2x mode in bf16):
  s[p,tau,e,c] = q[p,tau,c] * k[p,tau,e]        (DVE or GPSIMD, 1x: k bcast)
  E = exp(s)                                     (ACT)
  z[p,tau,c] = sum_e E    (pairwise tree over the middle axis, bf16 2x)
  w = v * 1/z                                    (small)
  P[p,tau,e,c] = E * w[bcast e]                  (DVE 2x)
  ctx[p,tau,e] = sum_c P  (pairwise tree over the last axis, bf16 2x)
PE does the QKV projections, x transposes and the output projection (all
bf16); ACT applies QKV biases (per-partition bias AP) while moving PSUM ->
SBUF bf16.
"""

import sys

sys.path.insert(0, "/opt/trn_rl_repo")

import numpy as np
from contextlib import ExitStack

import concourse.bass as bass
import concourse.bacc as bacc
import concourse.mybir as mybir
import concourse.tile as tile
from concourse.masks import make_identity

F32 = mybir.dt.float32
BF16 = mybir.dt.bfloat16
AF = mybir.ActivationFunctionType
ALU = mybir.AluOpType
AX = mybir.AxisListType

C = 32          # channels
K = 64          # per-channel width (also Wout dim)
F = C * K       # hidden = 2048
N_CORES = 8


def build_nc(T=512, Tc=8, flush=8, gp_mod=0):
    """Build the single-core Bass module for T tokens.

    Tc: tau-chunk size for the attention loop (free size per elementwise op
    = Tc*1024).
    flush: chunks per output-DMA flush group.
    gp_mod: if nonzero, chunks with ch % gp_mod == 0 run their score
    outer-product on GPSIMD (it shares SBUF ports with DVE, so concurrent
    use slows both; 0 = everything on DVE).
    """
    T2 = 2
    TH = T // T2            # tokens per half (tau range)
    NFC = F // 128          # 16 feature chunks of x
    NTT = (T + 127) // 128  # token tiles of x
    NCH = TH // Tc          # attention chunks
    NSUB = Tc // 4          # out-proj sub-blocks (4 taus x 32 m = 128 rows)
    assert TH % Tc == 0 and NCH % flush == 0 and Tc % 4 == 0

    nc = bacc.Bacc()
    x_d = nc.dram_tensor("x", [T, F], BF16, kind="ExternalInput")
    wq_d = nc.dram_tensor("wq", [F, K], BF16, kind="ExternalInput")
    wk_d = nc.dram_tensor("wk", [F, K], BF16, kind="ExternalInput")
    wv_d = nc.dram_tensor("wv", [F, K], BF16, kind="ExternalInput")
    bq_d = nc.dram_tensor("bq", [128, C], F32, kind="ExternalInput")
    bk_d = nc.dram_tensor("bk", [128, C], F32, kind="ExternalInput")
    bv_d = nc.dram_tensor("bv", [128, C], F32, kind="ExternalInput")
    wo_d = nc.dram_tensor("wout", [128, K], BF16, kind="ExternalInput")
    bo_d = nc.dram_tensor("bout", [128, K], BF16, kind="ExternalInput")
    out_d = nc.dram_tensor("out", [T, F], F32, kind="ExternalOutput")

    with ExitStack() as octx, nc.allow_low_precision(reason="bf16 attention"):
        tc = octx.enter_context(tile.TileContext(nc))
        const_pool = octx.enter_context(tc.tile_pool(name="const", bufs=1))
        qkv_pool = octx.enter_context(tc.tile_pool(name="qkv", bufs=1))

        ident = const_pool.tile([128, 128], BF16)
        make_identity(nc, ident[:])
        ones_row = const_pool.tile([1, 128], BF16)
        nc.gpsimd.memset(ones_row[:], 1.0)

        bq_sb = const_pool.tile([128, C], F32)
        bk_sb = const_pool.tile([128, C], F32)
        bv_sb = const_pool.tile([128, C], F32)
        wo_sb = const_pool.tile([128, K], BF16)
        bo_sb = const_pool.tile([128, K], BF16)
        nc.sync.dma_start(bq_sb[:], bq_d[:])
        nc.sync.dma_start(bk_sb[:], bk_d[:])
        nc.sync.dma_start(bv_sb[:], bv_d[:])
        nc.sync.dma_start(wo_sb[:], wo_d[:])
        nc.sync.dma_start(bo_sb[:], bo_d[:])

        # q is pre-scaled by 1/sqrt(K) on the host (weights and bias).
        # k is stored pair-doubled ([..., e, 2] with both slots equal) so the
        # score outer-product's k operand has a stride-1 last dim and the op
        # hits the DVE bf16 2x mode.
        qs_sb = qkv_pool.tile([128, TH, C], BF16)
        k_sb = qkv_pool.tile([128, TH, C, 2], BF16)
        v_sb = qkv_pool.tile([128, TH, C], BF16)

        # ---------------- phase 1: load x, transpose, QKV projections ----
        with (
            tc.tile_pool(name="xload", bufs=2) as xpool,
            tc.tile_pool(name="xt", bufs=1) as xtpool,
            tc.tile_pool(name="wgt", bufs=1) as wpool,
            tc.tile_pool(name="ps_qkv", bufs=6, space="PSUM") as ps1,
            tc.tile_pool(name="ps_tr", bufs=2, space="PSUM") as pst,
        ):
            wq_sb = wpool.tile([128, NFC, K], BF16)
            wk_sb = wpool.tile([128, NFC, K], BF16)
            wv_sb = wpool.tile([128, NFC, K], BF16)
            nc.sync.dma_start(wq_sb[:], wq_d[:].rearrange("(fc p) d -> p fc d", p=128))
            nc.sync.dma_start(wk_sb[:], wk_d[:].rearrange("(fc p) d -> p fc d", p=128))
            nc.sync.dma_start(wv_sb[:], wv_d[:].rearrange("(fc p) d -> p fc d", p=128))

            # xT[feat, token] per 128-feature chunk, via PE transpose.
            # Token-tile order 0,2,1,3 puts both t2-halves of the earliest
            # taus first so the attention loop can start sooner.
            xt_sb = xtpool.tile([128, NFC, T], BF16)
            tt_order = [0, 2, 1, 3] if NTT == 4 else list(range(NTT))
            for tt in tt_order:
                trows = min(128, T - tt * 128)
                x_sb = xpool.tile([128, F], BF16)
                for fc in range(NFC):
                    nc.sync.dma_start(
                        x_sb[:trows, fc * 128 : (fc + 1) * 128],
                        x_d[tt * 128 : tt * 128 + trows, fc * 128 : (fc + 1) * 128],
                    )
                for fc in range(NFC):
                    ps_t = pst.tile([128, 128], BF16)
                    nc.tensor.transpose(
                        ps_t[:, :trows],
                        x_sb[:trows, fc * 128 : (fc + 1) * 128],
                        ident[:trows, :trows],
                    )
                    nc.vector.tensor_copy(
                        xt_sb[:, fc, tt * 128 : tt * 128 + trows], ps_t[:, :trows]
                    )

            # per-channel QKV projections, split into tau-halves so the
            # attention loop can start after the first half is done
            NQ = 2
            HH = TH // NQ
            for half in range(NQ):
                hsl = slice(half * HH, half * HH + HH)
                for c in range(C):
                    fc, h = divmod(c, 2)
                    hp = slice(64 * h, 64 * h + 64)
                    for qkv_i, (w_sb, b_sb) in enumerate(
                        (
                            (wq_sb, bq_sb),
                            (wk_sb, bk_sb),
                            (wv_sb, bv_sb),
                        )
                    ):
                        ps = ps1.tile([128, HH], F32)
                        for t2 in range(T2):
                            nc.tensor.matmul(
                                ps[64 * t2 : 64 * t2 + 64, :],
                                w_sb[hp, fc, :],
                                xt_sb[
                                    hp, fc, t2 * TH + half * HH : t2 * TH + half * HH + HH
                                ],
                                start=True,
                                stop=True,
                            )
                        # bias + cast to bf16 on ACT (per-partition bias AP);
                        # k is written twice (pair-doubled storage)
                        if qkv_i == 1:
                            dsts = [k_sb[:, hsl, c, 0], k_sb[:, hsl, c, 1]]
                        else:
                            dsts = [(qs_sb if qkv_i == 0 else v_sb)[:, hsl, c]]
                        for dst in dsts:
                            nc.scalar.activation(
                                dst,
                                ps[:],
                                AF.Identity,
                                bias=b_sb[:, c : c + 1],
                            )

        # ---------------- phase 2: criss-cross attention + out-proj ------
        with (
            tc.tile_pool(name="s", bufs=3) as s_pool,
            tc.tile_pool(name="e", bufs=2) as e_pool,
            tc.tile_pool(name="zscr", bufs=2) as zscr_pool,
            tc.tile_pool(name="zsm", bufs=8) as z_pool,
            tc.tile_pool(name="ctx", bufs=4) as ctx_pool,
            tc.tile_pool(name="stage", bufs=2) as stage_pool,
            tc.tile_pool(name="ps_o", bufs=6, space="PSUM") as ps2,
        ):
            stage = [None, None]
            for ch in range(NCH):
                g = ch % flush
                if g == 0:
                    stage = [
                        stage_pool.tile(
                            [128, flush, NSUB, K], F32, tag="st0", name="st0"
                        ),
                        stage_pool.tile(
                            [128, flush, NSUB, K], F32, tag="st1", name="st1"
                        ),
                    ]
                tsl = slice(ch * Tc, (ch + 1) * Tc)
                on_gp = gp_mod and (ch % gp_mod == 0)
                ew = nc.gpsimd if on_gp else nc.vector

                # s[p, tau, e, c2, 2] = q[p, tau, (c2,2)] * k[p, tau, e]
                # (pair-split c so every operand has a stride-1 last dim ->
                # DVE bf16 2x mode)
                qs4 = (
                    qs_sb[:, tsl, :]
                    .rearrange("p t (c2 two) -> p t c2 two", two=2)
                    .unsqueeze(2)
                    .broadcast_to([128, Tc, C, C // 2, 2])
                )
                k4 = (
                    k_sb[:, tsl, :, :]
                    .unsqueeze(3)
                    .broadcast_to([128, Tc, C, C // 2, 2])
                )
                s_t = s_pool.tile([128, Tc, C, C], BF16)
                s5 = s_t[:].rearrange("p t e (c2 two) -> p t e c2 two", two=2)
                ew.tensor_tensor(s5, qs4, k4, ALU.mult)

                e_t = e_pool.tile([128, Tc, C, C], BF16)
                nc.scalar.activation(e_t[:], s_t[:], AF.Exp)

                # z[p, tau, c] = sum_e E: pairwise tree over the middle axis,
                # first level into scratch (E must survive for P), last level
                # to fp32 for the reciprocal.
                zscr = zscr_pool.tile([128, Tc, C // 2, C], BF16)
                z_t = z_pool.tile([128, Tc, C], F32, tag="z")
                nc.vector.tensor_tensor(
                    zscr[:], e_t[:, :, 0 : C // 2, :], e_t[:, :, C // 2 : C, :],
                    ALU.add,
                )
                width = C // 2
                while width > 1:
                    half = width // 2
                    lo = zscr[:, :, 0:half, :]
                    hi = zscr[:, :, half:width, :]
                    if half == 1:
                        nc.vector.tensor_tensor(
                            z_t[:], lo.squeeze(2), hi.squeeze(2), ALU.add
                        )
                    else:
                        nc.vector.tensor_tensor(lo, lo, hi, ALU.add)
                    width = half

                zi_t = z_pool.tile([128, Tc, C], F32, tag="zi")
                nc.vector.reciprocal_approx_fast(zi_t[:], z_t[:])
                w_t = z_pool.tile([128, Tc, C], BF16, tag="w")
                nc.vector.tensor_tensor(w_t[:], v_sb[:, tsl, :], zi_t[:], ALU.mult)

                # P[p, tau, e, c] = E * w (both packed -> DVE 2x), into s_t
                w4 = w_t[:].unsqueeze(2).broadcast_to([128, Tc, C, C])
                nc.vector.tensor_tensor(s_t[:], e_t[:], w4, ALU.mult)

                # ctx[p, tau, e] = sum_c P: pairwise tree over the last axis
                ctx_t = ctx_pool.tile([128, Tc, C], BF16, tag="ctx", name="ctx")
                width = C
                while width > 1:
                    half = width // 2
                    lo = s_t[:, :, :, 0:half]
                    hi = s_t[:, :, :, half:width]
                    if half == 1:
                        nc.vector.tensor_tensor(
                            ctx_t[:], lo.squeeze(3), hi.squeeze(3), ALU.add
                        )
                    else:
                        nc.vector.tensor_tensor(lo, lo, hi, ALU.add)
                    width = half

                # out-proj: out[(tau4,m), o] = sum_d ctx[(t2,d),(tau,m)] * Wout[d, o]
                # bout folded in via a K=1 accumulating matmul of ones x bout
                for t2 in range(T2):
                    dp = slice(64 * t2, 64 * t2 + 64)
                    for sub in range(NSUB):
                        ssl = slice(sub * 4, sub * 4 + 4)
                        po = ps2.tile([128, K], F32)
                        nc.tensor.matmul(
                            po[:],
                            ctx_t[dp, ssl, :].rearrange("p t c -> p (t c)"),
                            wo_sb[dp, :],
                            start=True,
                            stop=False,
                        )
                        nc.tensor.matmul(
                            po[:],
                            ones_row[0:1, 0:128],
                            bo_sb[0:1, :],
                            start=False,
                            stop=True,
                        )
                        nc.scalar.copy(stage[t2][:, g, sub, :], po[:])

                if g == flush - 1:
                    chb = ch // flush
                    ov = out_d[:].rearrange(
                        "(t2 chb chs sub tau) (m o) -> t2 chb tau m chs sub o",
                        t2=T2,
                        chb=NCH // flush,
                        chs=flush,
                        sub=NSUB,
                        tau=4,
                        m=C,
                    )
                    for t2 in range(T2):
                        nc.sync.dma_start(ov[t2, chb], stage[t2][:])

    nc.compile()
    return nc


def _bf16(a):
    import ml_dtypes

    return np.ascontiguousarray(a.astype(ml_dtypes.bfloat16))


def _host_prep(x, Wqkv, bqkv, Wout, bout):
    x = np.ascontiguousarray(np.asarray(x, dtype=np.float32)).reshape(-1, F)
    Wqkv = np.asarray(Wqkv, dtype=np.float32)
    bqkv = np.asarray(bqkv, dtype=np.float32)
    Wout = np.asarray(Wout, dtype=np.float32)
    bout = np.asarray(bout, dtype=np.float32)
    scale = 1.0 / np.sqrt(K)

    common = {
        "wq": _bf16((Wqkv[:, :, :K] * scale).reshape(F, K)),
        "wk": _bf16(Wqkv[:, :, K : 2 * K].reshape(F, K)),
        "wv": _bf16(Wqkv[:, :, 2 * K :].reshape(F, K)),
        "bq": np.ascontiguousarray(np.tile((bqkv[:, :K] * scale).T, (2, 1))),
        "bk": np.ascontiguousarray(np.tile(bqkv[:, K : 2 * K].T, (2, 1))),
        "bv": np.ascontiguousarray(np.tile(bqkv[:, 2 * K :].T, (2, 1))),
        "wout": _bf16(np.tile(Wout, (2, 1))),
        "bout": _bf16(np.tile(bout[None, :], (128, 1))),
    }
    return _bf16(x), common


_NC_CACHE = {}


def _get_nc(T):
    if T not in _NC_CACHE:
        _NC_CACHE[T] = build_nc(T=T)
    return _NC_CACHE[T]


def kernel(x, Wqkv, bqkv, Wout, bout, _trace=False):
    from concourse.bass_utils import run_bass_kernel_spmd

    xs, common = _host_prep(x, Wqkv, bqkv, Wout, bout)
    n_tok = xs.shape[0]
    tpc = n_tok // N_CORES
    in_maps = [
        {**common, "x": np.ascontiguousarray(xs[i * tpc : (i + 1) * tpc])}
        for i in range(N_CORES)
    ]
    nc = _get_nc(tpc)
    res = run_bass_kernel_spmd(nc, in_maps, list(range(N_CORES)), trace=_trace)
    out = np.concatenate([res.results[i]["out"] for i in range(N_CORES)], axis=0)
    out = out.reshape(np.asarray(x).shape)
    if _trace:
        kernel.last_results = res
    return out
